# revision 10
# baseline (speedup 1.0000x reference)
# Trainium2 Bass kernel for dynamic-routing capsule layer (nn_Capsule).
#
# Math (per batch b):
#   u_hat[n,i,j] = sum_d u[n,d] W[d, i*16+j]
#   b=0; for it in 0..2:
#     c = softmax(b, axis=i)
#     o[i,j] = sum_n c[i,n] u_hat[n,i,j]
#     if it<2: o' = l2norm(o); b[i,n] = sum_j o'[i,j] u_hat[n,i,j]
#   out = squash(o)
#
# Cost-model-aware restructuring (PE issue floor ~29ns/matmul dominates):
#   sT[d,i]  = sum_n u[n,d] c[i,n]     (64 matmuls/iter, lhsT=u chunk, ap=32)
#   O[i,:]   = sT^T @ W                (2 wide matmuls, ap=512)
#   o[i,j]   = O[i,16i+j]              (DVE mask-mul + group reduce)
#   o'       = o * rsqrt(|o|^2)        (ACT Square accum + DVE Quake-rsqrt)
#   vT[d,i]  = sum_ij W^T[ij,d] blockdiag(o'^T)[ij,i]
#              (1 replicate matmul + DVE mask + 8 matmuls vs 64 matvecs)
#   bT[n,i]  = sum_d uT[d,n] vT[d,i]   (64 matmuls/iter, ap=32)
#   softmax over i in [n-part, i-free] layout (full-lane ACT Exp + DVE/Pool)
# iter0: c uniform -> sT0 = colsum(u)/32 comes free from the uT-evacuation
# accum_out; O0 rows identical.
#
# ACT uses only Exp/Square/Copy (one act-table set, single load, no ~1.3us
# reloads). l2norm + squash run on DVE via bit-hack rsqrt + Newton.
# All matmul operands bf16 (1 cyc/row at any width; fp32 u load halved by
# SWDGE bf16 cast; DMA cost is dst-byte-based).
#
# Sharding: data-parallel over batch B=32 across 8 cores (4 batches/core),
# W replicated. No collectives.

import numpy as np

N_CORES = 8
B, N, D = 32, 4096, 256
I_CAPS, J_DIM = 32, 16
ROUTINGS = 3
EPS = 1e-7
MAGIC = 0x5F3759DF  # rsqrt seed


def build_nc(b_loc=B // N_CORES, n=N, d=D, enable_asserts=False, reps=1):
    from contextlib import ExitStack

    import concourse.bass as bass  # noqa: F401
    import concourse.tile as tile
    from concourse import bacc, mybir
    from concourse.masks import make_identity
    import bass_rust

    def chain(insts):
        # same-engine ordering edges: keeps each psum accumulation group's
        # start=True member first without tc.tile_critical()
        for a, b2 in zip(insts[1:], insts[:-1]):
            bass_rust.add_dep_helper(a.ins, b2.ins, sync=False,
                                     reason="pack order")

    f32 = mybir.dt.float32
    bf16 = mybir.dt.bfloat16
    i32 = mybir.dt.int32
    AX = mybir.AxisListType
    OP = mybir.AluOpType
    ACTF = mybir.ActivationFunctionType

    NC = n // 128       # 32 token chunks of 128 (token = 32*p + c)
    DC = d // 128       # 2
    IJ = I_CAPS * J_DIM  # 512
    CPB = 16            # token chunks per psum bank in mm2
    NBK = NC // CPB     # 2 banks per routing iteration
    I = I_CAPS

    nc = bacc.Bacc("TRN2", target_bir_lowering=False, debug=False,
                   enable_asserts=enable_asserts)
    u_dram = nc.dram_tensor("u", [b_loc, n, d], f32, kind="ExternalInput").ap()
    w_dram = nc.dram_tensor("w", [1, d, IJ], f32, kind="ExternalInput").ap()
    wt_dram = nc.dram_tensor("wt_scratch", [IJ, d], bf16, kind="Internal").ap()
    out_dram = nc.dram_tensor("out", [b_loc, I_CAPS, J_DIM], f32,
                              kind="ExternalOutput").ap()

    with tile.TileContext(nc) as tc, ExitStack() as ctx:
        const_pool = ctx.enter_context(tc.tile_pool(name="const", bufs=1))
        u_pool = ctx.enter_context(tc.tile_pool(name="u", bufs=3))
        uT_pool = ctx.enter_context(tc.tile_pool(name="uT", bufs=3))
        e_pool = ctx.enter_context(tc.tile_pool(name="e", bufs=2))
        cT_pool = ctx.enter_context(tc.tile_pool(name="cT", bufs=2))
        small = ctx.enter_context(tc.tile_pool(name="small", bufs=2))
        tiny = ctx.enter_context(tc.tile_pool(name="tiny", bufs=2))
        psum = ctx.enter_context(tc.tile_pool(name="ps", bufs=1, space="PSUM"))

        # ---- constants ----
        ident = const_pool.tile([128, 128], f32, name="ident")
        make_identity(nc, ident[:])
        ident_bf = const_pool.tile([128, 128], bf16, name="ident_bf")
        nc.vector.tensor_copy(ident_bf[:], ident[:])

        # om extract mask: mask[i, e] = 1 iff e//16 == i  ([32, 512] f32)
        mask = const_pool.tile([I, IJ], f32, name="mask")
        nc.gpsimd.memset(mask[:], 0.0)
        nc.gpsimd.affine_select(
            out=mask[:], in_=mask[:], compare_op=OP.is_gt, fill=1.0,
            base=-(J_DIM - 1), pattern=[[1, IJ]], channel_multiplier=-J_DIM)
        nc.gpsimd.affine_select(
            out=mask[:], in_=mask[:], compare_op=OP.is_ge, fill=0.0,
            base=0, pattern=[[1, IJ]], channel_multiplier=-J_DIM)

        # E16r[j, q] = 1 iff q % 16 == j  ([16, 128] bf16): replicates
        # o'T across the 8 j-blocks of each 128-ij chunk
        E16r = const_pool.tile([16, 128], bf16, name="E16r")
        nc.vector.tensor_copy(
            E16r[:].rearrange("j (t q) -> j t q", t=8),
            ident_bf[0:16, 0:16].unsqueeze(1).broadcast_to([16, 8, 16]))

        # G[i8, q] = 1 iff q//16 == i8  ([8, 128] bf16)
        G_sb = const_pool.tile([8, 128], bf16, name="G_sb")
        nc.vector.tensor_copy(
            G_sb[:].rearrange("a (b j) -> a b j", b=8),
            ident_bf[0:8, 0:8].unsqueeze(2).broadcast_to([8, 8, 16]))

        # maskblk[q3, blk, i] = 1 iff i == 8*blk + q3//16  ([128, 4, 32] f32)
        maskblk = const_pool.tile([128, 4, I], f32, name="maskblk")
        nc.gpsimd.memset(maskblk[:], 0.0)
        mb_ps = psum.tile([128, 4, 8], f32, tag="bT", bufs=2, name="mb_ps")
        for blk in range(4):
            nc.tensor.matmul(mb_ps[:, blk, :], G_sb[:], ident_bf[0:8, 0:8],
                             start=True, stop=True)
        for blk in range(4):
            nc.vector.tensor_copy(
                maskblk[:, blk, 8 * blk:8 * blk + 8], mb_ps[:, blk, :])

        # ---- W natural (bf16 cast): w_sb[q, e, f] = W[128e+q, f] ----
        w_sb = const_pool.tile([128, DC, IJ], bf16, name="w_sb")
        nc.gpsimd.dma_start(w_sb[:], w_dram[0].rearrange("(e q) f -> q e f",
                                                         q=128))

        # ---- wT_sb[q3, blk, e*128+dd] = W[128e+dd, 128blk+q3] ----
        wT_sb = const_pool.tile([128, 4, d], bf16, name="wT_sb")
        for e in range(DC):
            wt_ps = psum.tile([128, 4, 128], bf16, tag="tr", bufs=2,
                              name=f"wtps_{e}")
            pack = [nc.tensor.matmul(
                wt_ps[:, blk, :], w_sb[:, e, blk * 128:(blk + 1) * 128],
                ident_bf[:], is_transpose=True, start=True, stop=True)
                for blk in range(4)]
            chain(pack)
            nc.vector.tensor_copy(wT_sb[:, :, e * 128:(e + 1) * 128], wt_ps[:])

        for rep in range(reps):
            _body(nc, tc, mybir, b_loc, n, d, NC, DC, IJ, CPB, NBK, I,
                  f32, bf16, i32, AX, OP, ACTF, u_dram, out_dram,
                  u_pool, uT_pool, e_pool, cT_pool, small, tiny, psum,
                  ident_bf, mask, E16r, maskblk, w_sb, wT_sb, rep, chain)

    nc.compile()
    return nc


def _rsqrt(nc, tiny, OP, f32, i32, bf16, nrm, name, newton=2, out_dt=None):
    """y ~= nrm^-0.5 on DVE: Quake seed + `newton` Newton steps. [P, 1]."""
    P = nrm.shape[0]
    ish = tiny.tile([P, 1], i32, tag="ish", name=f"ish_{name}")
    nc.vector.tensor_scalar(ish[:], nrm.bitcast(i32), 1, None,
                            op0=OP.logical_shift_right)
    y = tiny.tile([P, 1], f32, tag="y0", name=f"y0_{name}")
    nc.vector.tensor_scalar(y[:].bitcast(i32), ish[:], -1, MAGIC,
                            op0=OP.mult, op1=OP.add)
    for k in range(newton):
        t = tiny.tile([P, 1], f32, tag=f"nt{k}", name=f"nt{k}_{name}")
        nc.vector.tensor_mul(t[:], y[:], y[:])
        nc.vector.tensor_mul(t[:], t[:], nrm)
        nc.vector.tensor_scalar(t[:], t[:], -0.5, 1.5, op0=OP.mult,
                                op1=OP.add)
        yn = tiny.tile([P, 1], f32 if (k < newton - 1 or out_dt is None)
                       else out_dt, tag=f"yn{k}", name=f"yn{k}_{name}")
        nc.vector.tensor_mul(yn[:], t[:], y[:])
        y = yn
    return y


def _body(nc, tc, mybir, b_loc, n, d, NC, DC, IJ, CPB, NBK, I,
          f32, bf16, i32, AX, OP, ACTF, u_dram, out_dram,
          u_pool, uT_pool, e_pool, cT_pool, small, tiny, psum,
          ident_bf, mask, E16r, maskblk, w_sb, wT_sb, rep, chain):
    J = 16

    for b in range(b_loc):
        # ---- u load (f32 -> bf16 SWDGE cast): u_t[p, c, dd] = u[b, 32p+c, dd]
        u_t = u_pool.tile([128, NC, d], bf16, tag="u", name=f"u_{rep}_{b}")
        nc.gpsimd.dma_start(
            u_t[:], u_dram[b].rearrange("(p c) dd -> p c dd", c=NC))

        # ---- uT[q, e, 128c+t] = u_t[t, c, 128e+q] via PE transposes.
        # Evacuation carries accum_out: acc[:, e, g] sums each tile's
        # columns, giving colsum(u) for iteration 0's uniform routing.
        uT_t = uT_pool.tile([128, DC, n], bf16, tag="uT", name=f"uT_{rep}_{b}")
        acc = tiny.tile([128, DC, 4], f32, tag="acc", name=f"acc_{rep}_{b}")
        ti = 0
        for e in range(DC):
            for cg in range(0, NC, 8):
                tr_ps = psum.tile([128, 8 * 128], bf16, tag="tr", bufs=2,
                                  name=f"trps_{rep}_{b}_{e}_{cg}")
                pack = [nc.tensor.matmul(
                    tr_ps[:, k * 128:(k + 1) * 128],
                    u_t[:, cg + k, e * 128:(e + 1) * 128],
                    ident_bf[:], is_transpose=True,
                    start=(k == 0), stop=(k == 7))
                    for k in range(8)]
                chain(pack)
                dst = uT_t[:, e, cg * 128:(cg + 8) * 128]
                a = acc[:, e, cg // 8:cg // 8 + 1]
                if ti % 2 == 0:
                    nc.vector.tensor_scalar(dst, tr_ps[:], 1.0, 0.0,
                                            op0=OP.mult, op1=OP.add,
                                            accum_out=a)
                else:
                    nc.scalar.activation(dst, tr_ps[:], ACTF.Copy,
                                         accum_out=a)
                ti += 1

        # sT0[q, e] = colsum(u)/32, replicated to [128, e, 32] bf16
        s0 = tiny.tile([128, DC], f32, tag="s0", name=f"s0_{rep}_{b}")
        nc.vector.tensor_reduce(s0[:], acc[:], axis=AX.X, op=OP.add)
        s0b = tiny.tile([128, DC], bf16, tag="s0b", name=f"s0b_{rep}_{b}")
        nc.vector.tensor_scalar_mul(s0b[:], s0[:], 1.0 / I)
        sT0_rep = small.tile([128, DC, I], bf16, tag="sT",
                             name=f"sT0r_{rep}_{b}")
        nc.vector.tensor_copy(
            sT0_rep[:], s0b[:].unsqueeze(2).broadcast_to([128, DC, I]))

        cT = None
        for it in range(ROUTINGS := 3):
            it_ps = psum.tile([128, 256], f32, tag="it", bufs=2,
                              name=f"it_{rep}_{b}_{it}")
            # region map (f32 cols): 0:64 sT | 64:96 orep | 96:160 vT
            #   | 160:176 oT' (bf16-bitcast, partitions 0:16)
            O_ps = psum.tile([I, IJ], f32, tag="O", bufs=2,
                             name=f"O_{rep}_{b}_{it}")

            # ---- mm1: sT[d, i] = sum_n u[n, d] c[i, n] ----
            if cT is None:
                sT_sb = sT0_rep
            else:
                sT_r = it_ps[:, 0:64]
                for e in range(DC):
                    pack = [nc.tensor.matmul(
                        sT_r[:, e * I:(e + 1) * I],
                        u_t[:, c, e * 128:(e + 1) * 128], cT[:, c, :],
                        start=(c == 0), stop=(c == NC - 1))
                        for c in range(NC)]
                    chain(pack)
                sT_sb = small.tile([128, DC, I], bf16, tag="sT",
                                   name=f"sT_{rep}_{b}_{it}")
                nc.vector.tensor_copy(sT_sb[:], sT_r)

            # ---- O = S @ W  [32, 512] ----
            for e in range(DC):
                nc.tensor.matmul(O_ps[:], sT_sb[:, e, :], w_sb[:, e, :],
                                 start=(e == 0), stop=(e == DC - 1))

            # ---- o[i, j] = O[i, 16i+j]; nrm[i] = sum_j o^2 ----
            om_sb = small.tile([I, IJ], f32, tag="om", name=f"om_{rep}_{b}_{it}")
            nc.vector.tensor_mul(om_sb[:], O_ps[:], mask[:])
            o_sb = tiny.tile([I, J], f32, tag="o", name=f"o_{rep}_{b}_{it}")
            nc.vector.tensor_reduce(
                o_sb[:], om_sb[:].rearrange("p (i j) -> p j i", j=J),
                axis=AX.X, op=OP.add)
            sq = tiny.tile([I, J], f32, tag="sq", name=f"sq_{rep}_{b}_{it}")
            nrm = tiny.tile([I, 1], f32, tag="nrm", name=f"nrm_{rep}_{b}_{it}")
            nc.scalar.activation(sq[:], o_sb[:], ACTF.Square, accum_out=nrm[:])

            if it < 2:
                # ---- o' = o * rsqrt(nrm) (bf16) ----
                rr = _rsqrt(nc, tiny, OP, f32, i32, bf16, nrm[:],
                            f"{rep}_{b}_{it}", newton=1)
                op_sb = tiny.tile([I, J], bf16, tag="op",
                                  name=f"op_{rep}_{b}_{it}")
                nc.vector.tensor_scalar_mul(op_sb[:], o_sb[:], rr[:, 0:1])

                # ---- oT'[j, i] via PE transpose ----
                oTp_r = it_ps[0:16, 160:176].bitcast(bf16)
                nc.tensor.matmul(oTp_r, op_sb[:], ident_bf[0:I, 0:I],
                                 is_transpose=True, start=True, stop=True)
                oTp_sb = tiny.tile([16, I], bf16, tag="oTp",
                                   name=f"oTp_{rep}_{b}_{it}")
                nc.vector.tensor_copy(oTp_sb[:], oTp_r)

                # ---- otil[ij, i] = blockdiag(o'T): replicate + mask ----
                orep_r = it_ps[:, 64:96]
                nc.tensor.matmul(orep_r, E16r[:], oTp_sb[:],
                                 start=True, stop=True)
                otil_sb = tiny.tile([128, 4, I], bf16, tag="otil",
                                    name=f"otil_{rep}_{b}_{it}")
                nc.vector.tensor_mul(
                    otil_sb[:],
                    orep_r.unsqueeze(1).broadcast_to([128, 4, I]),
                    maskblk[:])

                # ---- vT[d, (e,i)] = sum_ij wT[ij, d] otil[ij, i] ----
                vT_r = it_ps[:, 96:160]
                for e in range(DC):
                    pack = [nc.tensor.matmul(
                        vT_r[:, e * I:(e + 1) * I],
                        wT_sb[:, blk, e * 128:(e + 1) * 128],
                        otil_sb[:, blk, :], start=(blk == 0), stop=(blk == 3))
                        for blk in range(4)]
                    chain(pack)
                vT_sb = tiny.tile([128, DC, I], bf16, tag="vT",
                                  name=f"vT_{rep}_{b}_{it}")
                nc.vector.tensor_copy(
                    vT_sb[:], vT_r.rearrange("q (e i) -> q e i", e=DC))

                # ---- mm2 + softmax over i in [n-part, i] layout ----
                e_sbT = e_pool.tile([128, NC, I], f32, tag="e",
                                    name=f"e_{rep}_{b}_{it}")
                z_sb = tiny.tile([128, NC], f32, tag="z",
                                 name=f"z_{rep}_{b}_{it}")
                r_sb = tiny.tile([128, NC], f32, tag="r",
                                 name=f"r_{rep}_{b}_{it}")
                cT = cT_pool.tile([128, NC, I], bf16, tag="cT",
                                  name=f"cT_{rep}_{b}_{it + 1}")
                for bank in range(NBK):
                    bT_ps = psum.tile([128, CPB, I], f32, tag="bT", bufs=2,
                                      name=f"bT_{rep}_{b}_{it}_{bank}")
                    for cc in range(CPB):
                        c = bank * CPB + cc
                        pack = [nc.tensor.matmul(
                            bT_ps[:, cc, :],
                            uT_t[:, e, c * 128:(c + 1) * 128],
                            vT_sb[:, e, :], start=(e == 0), stop=(e == DC - 1))
                            for e in range(DC)]
                        chain(pack)
                    bsl = slice(bank * CPB, (bank + 1) * CPB)
                    nc.scalar.activation(e_sbT[:, bsl, :], bT_ps[:], ACTF.Exp)
                    nc.vector.tensor_reduce(z_sb[:, bsl], e_sbT[:, bsl, :],
                                            axis=AX.X, op=OP.add)
                nc.vector.reciprocal(r_sb[:], z_sb[:])
                for bank in range(NBK):
                    bsl = slice(bank * CPB, (bank + 1) * CPB)
                    nc.gpsimd.tensor_mul(
                        cT[:, bsl, :], e_sbT[:, bsl, :],
                        r_sb[:, bsl].unsqueeze(2).broadcast_to(
                            [128, CPB, I]))
            else:
                # ---- squash: out = (sqrt(s2)/(0.5+s2)) * o, s2 = nrm+EPS ----
                s2 = tiny.tile([I, 1], f32, tag="s2", name=f"s2_{rep}_{b}")
                nc.vector.tensor_scalar_add(s2[:], nrm[:], EPS)
                ry = _rsqrt(nc, tiny, OP, f32, i32, bf16, s2[:],
                            f"sq_{rep}_{b}", newton=2)
                sqrt_s2 = tiny.tile([I, 1], f32, tag="sqs2",
                                    name=f"sqs2_{rep}_{b}")
                nc.vector.tensor_mul(sqrt_s2[:], ry[:], s2[:])
                den = tiny.tile([I, 1], f32, tag="den", name=f"den_{rep}_{b}")
                nc.vector.tensor_scalar_add(den[:], s2[:], 0.5)
                rden = tiny.tile([I, 1], f32, tag="rden",
                                 name=f"rden_{rep}_{b}")
                nc.vector.reciprocal(rden[:], den[:])
                scl = tiny.tile([I, 1], f32, tag="scl", name=f"scl_{rep}_{b}")
                nc.vector.tensor_mul(scl[:], sqrt_s2[:], rden[:])
                o_out = tiny.tile([I, J], f32, tag="oout",
                                  name=f"oout_{rep}_{b}")
                nc.vector.tensor_scalar_mul(o_out[:], o_sb[:], scl[:, 0:1])
                nc.sync.dma_start(out_dram[b], o_out[:])


_NC_CACHE = {}


def _get_nc():
    if "nc" not in _NC_CACHE:
        _NC_CACHE["nc"] = build_nc()
    return _NC_CACHE["nc"]


def kernel(u_vecs: np.ndarray, W: np.ndarray) -> np.ndarray:
    from concourse.bass_utils import run_bass_kernel_spmd

    u_vecs = np.ascontiguousarray(u_vecs, dtype=np.float32)
    W = np.ascontiguousarray(W, dtype=np.float32)
    b_loc = B // N_CORES
    nc = _get_nc()
    in_maps = [
        {"u": u_vecs[i * b_loc:(i + 1) * b_loc], "w": W}
        for i in range(N_CORES)
    ]
    res = run_bass_kernel_spmd(nc, in_maps, core_ids=list(range(N_CORES)))
    return np.concatenate([r["out"] for r in res.results], axis=0)


# revision 16
# speedup vs baseline: 1.0504x; 1.0504x over previous
# Trainium2 Bass kernel for dynamic-routing capsule layer (nn_Capsule).
#
# Math (per batch b):
#   u_hat[n,i,j] = sum_d u[n,d] W[d, i*16+j]
#   b=0; for it in 0..2:
#     c = softmax(b, axis=i)
#     o[i,j] = sum_n c[i,n] u_hat[n,i,j]
#     if it<2: o' = l2norm(o); b[i,n] = sum_j o'[i,j] u_hat[n,i,j]
#   out = squash(o)
#
# Cost-model-aware restructuring (PE issue floor ~29ns/matmul dominates):
#   sT[d,i]  = sum_n u[n,d] c[i,n]     (64 matmuls/iter, lhsT=u chunk, ap=32)
#   O[i,:]   = sT^T @ W                (2 wide matmuls, ap=512)
#   o[i,j]   = O[i,16i+j]              (DVE mask-mul + group reduce)
#   o'       = o * rsqrt(|o|^2)        (ACT Square accum + DVE Quake-rsqrt)
#   vT[d,i]  = sum_ij W^T[ij,d] blockdiag(o'^T)[ij,i]
#              (1 replicate matmul + DVE mask + 8 matmuls vs 64 matvecs)
#   bT[n,i]  = sum_d uT[d,n] vT[d,i]   (64 matmuls/iter, ap=32)
#   softmax over i in [n-part, i-free] layout (full-lane ACT Exp + DVE/Pool)
# iter0: c uniform -> sT0 = colsum(u)/32 comes free from the uT-evacuation
# accum_out; O0 rows identical.
#
# ACT uses only Exp/Square/Copy (one act-table set, single load, no ~1.3us
# reloads). l2norm + squash run on DVE via bit-hack rsqrt + Newton.
# All matmul operands bf16 (1 cyc/row at any width; fp32 u load halved by
# SWDGE bf16 cast; DMA cost is dst-byte-based).
#
# Sharding: data-parallel over batch B=32 across 8 cores (4 batches/core),
# W replicated. No collectives.

import numpy as np

N_CORES = 8
B, N, D = 32, 4096, 256
I_CAPS, J_DIM = 32, 16
ROUTINGS = 3
EPS = 1e-7
MAGIC = 0x5F3759DF  # rsqrt seed


def build_nc(b_loc=B // N_CORES, n=N, d=D, enable_asserts=False, reps=1):
    from contextlib import ExitStack

    import concourse.bass as bass  # noqa: F401
    import concourse.tile as tile
    from concourse import bacc, mybir
    from concourse.masks import make_identity
    import bass_rust

    def chain(insts):
        # same-engine ordering edges: keeps each psum accumulation group's
        # start=True member first without tc.tile_critical()
        for a, b2 in zip(insts[1:], insts[:-1]):
            bass_rust.add_dep_helper(a.ins, b2.ins, sync=False,
                                     reason="pack order")

    f32 = mybir.dt.float32
    bf16 = mybir.dt.bfloat16
    i32 = mybir.dt.int32
    AX = mybir.AxisListType
    OP = mybir.AluOpType
    ACTF = mybir.ActivationFunctionType

    NC = n // 128       # 32 token chunks of 128 (token = 32*p + c)
    DC = d // 128       # 2
    IJ = I_CAPS * J_DIM  # 512
    CPB = 16            # token chunks per psum bank in mm2
    NBK = NC // CPB     # 2 banks per routing iteration
    I = I_CAPS

    nc = bacc.Bacc("TRN2", target_bir_lowering=False, debug=False,
                   enable_asserts=enable_asserts)
    u_dram = nc.dram_tensor("u", [b_loc, n, d], f32, kind="ExternalInput").ap()
    w_dram = nc.dram_tensor("w", [1, d, IJ], f32, kind="ExternalInput").ap()
    wt_dram = nc.dram_tensor("wt_scratch", [IJ, d], bf16, kind="Internal").ap()
    out_dram = nc.dram_tensor("out", [b_loc, I_CAPS, J_DIM], f32,
                              kind="ExternalOutput").ap()

    with tile.TileContext(nc) as tc, ExitStack() as ctx:
        const_pool = ctx.enter_context(tc.tile_pool(name="const", bufs=1))
        u_pool = ctx.enter_context(tc.tile_pool(name="u", bufs=4))
        uT_pool = ctx.enter_context(tc.tile_pool(name="uT", bufs=3))
        e_pool = ctx.enter_context(tc.tile_pool(name="e", bufs=2))
        cT_pool = ctx.enter_context(tc.tile_pool(name="cT", bufs=2))
        small = ctx.enter_context(tc.tile_pool(name="small", bufs=2))
        tiny = ctx.enter_context(tc.tile_pool(name="tiny", bufs=2))
        psum = ctx.enter_context(tc.tile_pool(name="ps", bufs=1, space="PSUM"))

        # ---- constants ----
        ident = const_pool.tile([128, 128], f32, name="ident")
        make_identity(nc, ident[:])
        ident_bf = const_pool.tile([128, 128], bf16, name="ident_bf")
        nc.vector.tensor_copy(ident_bf[:], ident[:])

        # om extract mask: mask[i, e] = 1 iff e//16 == i  ([32, 512] f32)
        mask = const_pool.tile([I, IJ], f32, name="mask")
        nc.gpsimd.memset(mask[:], 0.0)
        nc.gpsimd.affine_select(
            out=mask[:], in_=mask[:], compare_op=OP.is_gt, fill=1.0,
            base=-(J_DIM - 1), pattern=[[1, IJ]], channel_multiplier=-J_DIM)
        nc.gpsimd.affine_select(
            out=mask[:], in_=mask[:], compare_op=OP.is_ge, fill=0.0,
            base=0, pattern=[[1, IJ]], channel_multiplier=-J_DIM)

        # E16r[j, q] = 1 iff q % 16 == j  ([16, 128] bf16): replicates
        # o'T across the 8 j-blocks of each 128-ij chunk
        E16r = const_pool.tile([16, 128], bf16, name="E16r")
        nc.vector.tensor_copy(
            E16r[:].rearrange("j (t q) -> j t q", t=8),
            ident_bf[0:16, 0:16].unsqueeze(1).broadcast_to([16, 8, 16]))

        # G[i8, q] = 1 iff q//16 == i8  ([8, 128] bf16)
        G_sb = const_pool.tile([8, 128], bf16, name="G_sb")
        nc.vector.tensor_copy(
            G_sb[:].rearrange("a (b j) -> a b j", b=8),
            ident_bf[0:8, 0:8].unsqueeze(2).broadcast_to([8, 8, 16]))

        # maskblk[q3, blk, i] = 1 iff i == 8*blk + q3//16  ([128, 4, 32] f32)
        maskblk = const_pool.tile([128, 4, I], f32, name="maskblk")
        nc.gpsimd.memset(maskblk[:], 0.0)
        mb_ps = psum.tile([128, 4, 8], f32, tag="bT", bufs=2, name="mb_ps")
        for blk in range(4):
            nc.tensor.matmul(mb_ps[:, blk, :], G_sb[:], ident_bf[0:8, 0:8],
                             start=True, stop=True)
        for blk in range(4):
            nc.vector.tensor_copy(
                maskblk[:, blk, 8 * blk:8 * blk + 8], mb_ps[:, blk, :])

        # ---- W natural (bf16 cast): w_sb[q, e, f] = W[128e+q, f] ----
        w_sb = const_pool.tile([128, DC, IJ], bf16, name="w_sb")
        nc.gpsimd.dma_start(w_sb[:], w_dram[0].rearrange("(e q) f -> q e f",
                                                         q=128))

        # ---- wT_sb[q3, blk, e*128+dd] = W[128e+dd, 128blk+q3] ----
        wT_sb = const_pool.tile([128, 4, d], bf16, name="wT_sb")
        for e in range(DC):
            wt_ps = psum.tile([128, 4, 128], bf16, tag="tr", bufs=2,
                              name=f"wtps_{e}")
            pack = [nc.tensor.matmul(
                wt_ps[:, blk, :], w_sb[:, e, blk * 128:(blk + 1) * 128],
                ident_bf[:], is_transpose=True, start=True, stop=True)
                for blk in range(4)]
            chain(pack)
            nc.vector.tensor_copy(wT_sb[:, :, e * 128:(e + 1) * 128], wt_ps[:])

        for rep in range(reps):
            _body(nc, tc, mybir, b_loc, n, d, NC, DC, IJ, CPB, NBK, I,
                  f32, bf16, i32, AX, OP, ACTF, u_dram, out_dram,
                  u_pool, uT_pool, e_pool, cT_pool, small, tiny, psum,
                  ident_bf, mask, E16r, maskblk, w_sb, wT_sb, rep, chain)

    nc.compile()
    return nc


def _rsqrt(nc, tiny, OP, f32, i32, bf16, nrm, name, newton=2, out_dt=None):
    """y ~= nrm^-0.5 on DVE: Quake seed + `newton` Newton steps. [P, 1]."""
    P = nrm.shape[0]
    ish = tiny.tile([P, 1], i32, tag="ish", name=f"ish_{name}")
    nc.vector.tensor_scalar(ish[:], nrm.bitcast(i32), 1, None,
                            op0=OP.logical_shift_right)
    y = tiny.tile([P, 1], f32, tag="y0", name=f"y0_{name}")
    nc.vector.tensor_scalar(y[:].bitcast(i32), ish[:], -1, MAGIC,
                            op0=OP.mult, op1=OP.add)
    for k in range(newton):
        t = tiny.tile([P, 1], f32, tag=f"nt{k}", name=f"nt{k}_{name}")
        nc.vector.tensor_mul(t[:], y[:], y[:])
        nc.vector.tensor_mul(t[:], t[:], nrm)
        nc.vector.tensor_scalar(t[:], t[:], -0.5, 1.5, op0=OP.mult,
                                op1=OP.add)
        yn = tiny.tile([P, 1], f32 if (k < newton - 1 or out_dt is None)
                       else out_dt, tag=f"yn{k}", name=f"yn{k}_{name}")
        nc.vector.tensor_mul(yn[:], t[:], y[:])
        y = yn
    return y


def _body(nc, tc, mybir, b_loc, n, d, NC, DC, IJ, CPB, NBK, I,
          f32, bf16, i32, AX, OP, ACTF, u_dram, out_dram,
          u_pool, uT_pool, e_pool, cT_pool, small, tiny, psum,
          ident_bf, mask, E16r, maskblk, w_sb, wT_sb, rep, chain):
    J = 16

    # ---- u loads prefetched for every batch (f32 -> bf16 SWDGE cast):
    # u_t[p, c, dd] = u[b, 32p+c, dd]. Issued up-front so Pool's in-order
    # queue never blocks the next batch's load behind this batch's softmax.
    u_ts = []
    for b in range(b_loc):
        u_t = u_pool.tile([128, NC, d], bf16, tag="u", name=f"u_{rep}_{b}")
        nc.gpsimd.dma_start(
            u_t[:], u_dram[b].rearrange("(p c) dd -> p c dd", c=NC))
        u_ts.append(u_t)

    for b in range(b_loc):
        u_t = u_ts[b]
        # ---- uT[q, e, 128c+t] = u_t[t, c, 128e+q] via PE transposes.
        # Evacuation carries accum_out: acc[:, e, g] sums each tile's
        # columns, giving colsum(u) for iteration 0's uniform routing.
        uT_t = uT_pool.tile([128, DC, n], bf16, tag="uT", name=f"uT_{rep}_{b}")
        acc = tiny.tile([128, DC, 4], f32, tag="acc", name=f"acc_{rep}_{b}")
        ti = 0
        for e in range(DC):
            for cg in range(0, NC, 8):
                tr_ps = psum.tile([128, 8 * 128], bf16, tag="tr", bufs=2,
                                  name=f"trps_{rep}_{b}_{e}_{cg}")
                pack = [nc.tensor.matmul(
                    tr_ps[:, k * 128:(k + 1) * 128],
                    u_t[:, cg + k, e * 128:(e + 1) * 128],
                    ident_bf[:], is_transpose=True,
                    start=(k == 0), stop=(k == 7))
                    for k in range(8)]
                chain(pack)
                dst = uT_t[:, e, cg * 128:(cg + 8) * 128]
                a = acc[:, e, cg // 8:cg // 8 + 1]
                if ti % 2 == 0:
                    nc.vector.tensor_scalar(dst, tr_ps[:], 1.0, 0.0,
                                            op0=OP.mult, op1=OP.add,
                                            accum_out=a)
                else:
                    nc.scalar.activation(dst, tr_ps[:], ACTF.Copy,
                                         accum_out=a)
                ti += 1

        # sT0[q, e] = colsum(u)/32, replicated to [128, e, 32] bf16
        s0 = tiny.tile([128, DC], f32, tag="s0", name=f"s0_{rep}_{b}")
        nc.vector.tensor_reduce(s0[:], acc[:], axis=AX.X, op=OP.add)
        s0b = tiny.tile([128, DC], bf16, tag="s0b", name=f"s0b_{rep}_{b}")
        nc.vector.tensor_scalar_mul(s0b[:], s0[:], 1.0 / I)
        sT0_rep = small.tile([128, DC, I], bf16, tag="sT",
                             name=f"sT0r_{rep}_{b}")
        nc.vector.tensor_copy(
            sT0_rep[:], s0b[:].unsqueeze(2).broadcast_to([128, DC, I]))

        cT = None
        for it in range(ROUTINGS := 3):
            it_ps = psum.tile([128, 256], f32, tag="it", bufs=2,
                              name=f"it_{rep}_{b}_{it}")
            # region map (f32 cols): 0:64 sT | 64:96 orep | 96:160 vT
            #   | 160:176 oT' (bf16-bitcast, partitions 0:16)
            O_ps = psum.tile([I, IJ], f32, tag="O", bufs=2,
                             name=f"O_{rep}_{b}_{it}")

            # ---- mm1: sT[d, i] = sum_n u[n, d] c[i, n] ----
            if cT is None:
                sT_sb = sT0_rep
            else:
                sT_r = it_ps[:, 0:64]
                for e in range(DC):
                    pack = [nc.tensor.matmul(
                        sT_r[:, e * I:(e + 1) * I],
                        u_t[:, c, e * 128:(e + 1) * 128], cT[:, c, :],
                        start=(c == 0), stop=(c == NC - 1))
                        for c in range(NC)]
                    chain(pack)
                sT_sb = small.tile([128, DC, I], bf16, tag="sT",
                                   name=f"sT_{rep}_{b}_{it}")
                for e in range(DC):
                    nc.vector.tensor_copy(sT_sb[:, e, :],
                                          sT_r[:, e * I:(e + 1) * I])

            # ---- O = S @ W  [32, 512] ----
            for e in range(DC):
                nc.tensor.matmul(O_ps[:], sT_sb[:, e, :], w_sb[:, e, :],
                                 start=(e == 0), stop=(e == DC - 1))

            # ---- o[i, j] = O[i, 16i+j]; nrm[i] = sum_j o^2 ----
            om_sb = small.tile([I, IJ], f32, tag="om", name=f"om_{rep}_{b}_{it}")
            nc.vector.tensor_mul(om_sb[:], O_ps[:], mask[:])
            o_sb = tiny.tile([I, J], f32, tag="o", name=f"o_{rep}_{b}_{it}")
            nc.vector.tensor_reduce(
                o_sb[:], om_sb[:].rearrange("p (i j) -> p j i", j=J),
                axis=AX.X, op=OP.add)
            sq = tiny.tile([I, J], f32, tag="sq", name=f"sq_{rep}_{b}_{it}")
            nrm = tiny.tile([I, 1], f32, tag="nrm", name=f"nrm_{rep}_{b}_{it}")
            nc.scalar.activation(sq[:], o_sb[:], ACTF.Square, accum_out=nrm[:])

            if it < 2:
                # ---- o' = o * rsqrt(nrm) (bf16) ----
                rr = _rsqrt(nc, tiny, OP, f32, i32, bf16, nrm[:],
                            f"{rep}_{b}_{it}", newton=1)
                op_sb = tiny.tile([I, J], bf16, tag="op",
                                  name=f"op_{rep}_{b}_{it}")
                nc.vector.tensor_scalar_mul(op_sb[:], o_sb[:], rr[:, 0:1])

                # ---- oT'[j, i] via PE transpose ----
                oTp_r = it_ps[0:16, 160:176].bitcast(bf16)
                nc.tensor.matmul(oTp_r, op_sb[:], ident_bf[0:I, 0:I],
                                 is_transpose=True, start=True, stop=True)
                oTp_sb = tiny.tile([16, I], bf16, tag="oTp",
                                   name=f"oTp_{rep}_{b}_{it}")
                nc.vector.tensor_copy(oTp_sb[:], oTp_r)

                # ---- otil[ij, i] = blockdiag(o'T): replicate + mask ----
                orep_r = it_ps[:, 64:96]
                nc.tensor.matmul(orep_r, E16r[:], oTp_sb[:],
                                 start=True, stop=True)
                otil_sb = tiny.tile([128, 4, I], bf16, tag="otil",
                                    name=f"otil_{rep}_{b}_{it}")
                nc.vector.tensor_mul(
                    otil_sb[:],
                    orep_r.unsqueeze(1).broadcast_to([128, 4, I]),
                    maskblk[:])

                # ---- vT[d, (e,i)] = sum_ij wT[ij, d] otil[ij, i] ----
                vT_r = it_ps[:, 96:160]
                for e in range(DC):
                    pack = [nc.tensor.matmul(
                        vT_r[:, e * I:(e + 1) * I],
                        wT_sb[:, blk, e * 128:(e + 1) * 128],
                        otil_sb[:, blk, :], start=(blk == 0), stop=(blk == 3))
                        for blk in range(4)]
                    chain(pack)
                vT_sb = tiny.tile([128, DC, I], bf16, tag="vT",
                                  name=f"vT_{rep}_{b}_{it}")
                nc.vector.tensor_copy(
                    vT_sb[:], vT_r.rearrange("q (e i) -> q e i", e=DC))

                # ---- mm2 + softmax over i in [n-part, i] layout ----
                e_sbT = e_pool.tile([128, NC, I], f32, tag="e",
                                    name=f"e_{rep}_{b}_{it}")
                z_sb = tiny.tile([128, NC], f32, tag="z",
                                 name=f"z_{rep}_{b}_{it}")
                r_sb = tiny.tile([128, NC], f32, tag="r",
                                 name=f"r_{rep}_{b}_{it}")
                cT = cT_pool.tile([128, NC, I], bf16, tag="cT",
                                  name=f"cT_{rep}_{b}_{it + 1}")
                for bank in range(NBK):
                    bT_ps = psum.tile([128, CPB, I], f32, tag="bT", bufs=2,
                                      name=f"bT_{rep}_{b}_{it}_{bank}")
                    for cc in range(CPB):
                        c = bank * CPB + cc
                        pack = [nc.tensor.matmul(
                            bT_ps[:, cc, :],
                            uT_t[:, e, c * 128:(c + 1) * 128],
                            vT_sb[:, e, :], start=(e == 0), stop=(e == DC - 1))
                            for e in range(DC)]
                        chain(pack)
                    bsl = slice(bank * CPB, (bank + 1) * CPB)
                    nc.scalar.activation(e_sbT[:, bsl, :], bT_ps[:], ACTF.Exp)
                    nc.vector.tensor_reduce(z_sb[:, bsl], e_sbT[:, bsl, :],
                                            axis=AX.X, op=OP.add)
                    nc.vector.reciprocal(r_sb[:, bsl], z_sb[:, bsl])
                    nc.gpsimd.tensor_mul(
                        cT[:, bsl, :], e_sbT[:, bsl, :],
                        r_sb[:, bsl].unsqueeze(2).broadcast_to(
                            [128, CPB, I]))
            else:
                # ---- squash: out = (sqrt(s2)/(0.5+s2)) * o, s2 = nrm+EPS ----
                s2 = tiny.tile([I, 1], f32, tag="s2", name=f"s2_{rep}_{b}")
                nc.vector.tensor_scalar_add(s2[:], nrm[:], EPS)
                ry = _rsqrt(nc, tiny, OP, f32, i32, bf16, s2[:],
                            f"sq_{rep}_{b}", newton=2)
                sqrt_s2 = tiny.tile([I, 1], f32, tag="sqs2",
                                    name=f"sqs2_{rep}_{b}")
                nc.vector.tensor_mul(sqrt_s2[:], ry[:], s2[:])
                den = tiny.tile([I, 1], f32, tag="den", name=f"den_{rep}_{b}")
                nc.vector.tensor_scalar_add(den[:], s2[:], 0.5)
                rden = tiny.tile([I, 1], f32, tag="rden",
                                 name=f"rden_{rep}_{b}")
                nc.vector.reciprocal(rden[:], den[:])
                scl = tiny.tile([I, 1], f32, tag="scl", name=f"scl_{rep}_{b}")
                nc.vector.tensor_mul(scl[:], sqrt_s2[:], rden[:])
                o_out = tiny.tile([I, J], f32, tag="oout",
                                  name=f"oout_{rep}_{b}")
                nc.vector.tensor_scalar_mul(o_out[:], o_sb[:], scl[:, 0:1])
                nc.sync.dma_start(out_dram[b], o_out[:])


_NC_CACHE = {}


def _get_nc():
    if "nc" not in _NC_CACHE:
        _NC_CACHE["nc"] = build_nc()
    return _NC_CACHE["nc"]


def kernel(u_vecs: np.ndarray, W: np.ndarray) -> np.ndarray:
    from concourse.bass_utils import run_bass_kernel_spmd

    u_vecs = np.ascontiguousarray(u_vecs, dtype=np.float32)
    W = np.ascontiguousarray(W, dtype=np.float32)
    b_loc = B // N_CORES
    nc = _get_nc()
    in_maps = [
        {"u": u_vecs[i * b_loc:(i + 1) * b_loc], "w": W}
        for i in range(N_CORES)
    ]
    res = run_bass_kernel_spmd(nc, in_maps, core_ids=list(range(N_CORES)))
    return np.concatenate([r["out"] for r in res.results], axis=0)
